# revision 22
# baseline (speedup 1.0000x reference)
"""RBF-kernel attention (sparse_attention nn_Attention_31155692765862) on 8 trn2 cores.

Head-parallel sharding: core h computes head h for all batches:
    Q = x @ W_q[h], K = x @ W_k[h], V = x @ W_v[h]
    attn = exp(-gamma_h * max(q2 + k2 - 2 Q K^T, 0))
    y_h  = (attn @ V) @ W_o[:, h*E:(h+1)*E].T          (partial over heads)
Host sums the 8 partial y_h.

Device math (per batch b), all matmuls bf16 inputs with fp32 PSUM accum:
    QT[f,s] = sum_e Wq[e,f] xT[e,s]          (lhsT=Wq chunk, rhs=xT chunk)
    KT[f,s] likewise; V[t,f] = sum_e xT[e,t] Wv[e,f]
    q2[s] = sum_f QT[f,s]^2  via Square-eviction + ones-matmul (output [128,1])
    ST[t,s] = sum_f KT[f,t] QT[f,s]
    A[t,s] = exp(2g*ST - g*k2[t])            (ACT exp, per-partition bias)
    OT[f,s] = sum_t V[t,f] A[t,s]
    y[s,d] = (sum_f OT[f,s] WoT[f,d]) * exp(-g*q2[s])  (scale on eviction)
The relu clamp max(d2,0) is dropped: d2 < 0 only from rounding (|d2| ~ 1e-3
relative), so exp(-g*d2) differs from exp(0) by < 1e-4 relative.
"""

import numpy as np
import ml_dtypes
from contextlib import ExitStack

B, S, E, H, P = 4, 1024, 512, 8, 128
EC = E // P  # contraction chunks of 128 over the embedding dim
FC = E // P  # feature chunks of 128
TC = S // P  # row chunks of 128 over the sequence dim
NF = 512     # matmul moving free dim / PSUM bank width (fp32)
SH = S // NF  # s halves per row

_CACHE = {}


def _build_nc():
    import concourse.tile as tile
    from concourse import bacc, mybir

    bf16 = mybir.dt.bfloat16
    f32 = mybir.dt.float32
    Square = mybir.ActivationFunctionType.Square
    Exp = mybir.ActivationFunctionType.Exp
    Copy = mybir.ActivationFunctionType.Copy

    nc = bacc.Bacc("TRN2", target_bir_lowering=False, debug=False)

    xT_d = nc.dram_tensor("xT", [B * E, S], bf16, kind="ExternalInput")
    wq_d = nc.dram_tensor("wq", [E, E], bf16, kind="ExternalInput")
    wk_d = nc.dram_tensor("wk", [E, E], bf16, kind="ExternalInput")
    wv_d = nc.dram_tensor("wv", [E, E], bf16, kind="ExternalInput")
    woT_d = nc.dram_tensor("woT", [E, E], bf16, kind="ExternalInput")
    gam_d = nc.dram_tensor("gam", [1, 1], f32, kind="ExternalInput")
    y_d = nc.dram_tensor("y", [B * S, E], f32, kind="ExternalOutput")

    with tile.TileContext(nc) as tc, ExitStack() as ctx:
        consts = ctx.enter_context(tc.tile_pool(name="consts", bufs=1))
        wpool = ctx.enter_context(tc.tile_pool(name="wpool", bufs=1))
        xpool = ctx.enter_context(tc.tile_pool(name="xpool", bufs=2))
        qkpool = ctx.enter_context(tc.tile_pool(name="qkpool", bufs=2))
        sqpool = ctx.enter_context(tc.tile_pool(name="sqpool", bufs=2))
        vpool = ctx.enter_context(tc.tile_pool(name="vpool", bufs=2))
        apool = ctx.enter_context(tc.tile_pool(name="apool", bufs=2))
        opool = ctx.enter_context(tc.tile_pool(name="opool", bufs=2))
        ypool = ctx.enter_context(tc.tile_pool(name="ypool", bufs=4))
        smalls = ctx.enter_context(tc.tile_pool(name="smalls", bufs=2))
        psum = ctx.enter_context(tc.tile_pool(name="psum", bufs=8, space="PSUM"))

        # PE warmup: dummy matmuls on a zeroed tile bridge the initial DMA
        # wait and release the HAM clock throttle before real matmuls start.
        # Memsets go on GpSimd so nothing upstream delays them.
        warm_src = consts.tile([P, NF], bf16, tag="warm_src")
        nc.gpsimd.memset(warm_src, 0.0)
        ones = consts.tile([P, 1], bf16, tag="ones")
        nc.gpsimd.memset(ones, 1.0)
        warm_ps = psum.tile([P, NF], f32, tag="ps", name="warm_ps")
        for _ in range(6):
            nc.tensor.matmul(warm_ps, warm_src[:, :P], warm_src, start=True, stop=True)

        # Weights: [p, feat] per 128-row chunk of the embedding dim. wq is
        # split per chunk and interleaved with batch-0 x chunks so the first
        # projection's accumulation chain starts as soon as chunk 0 lands.
        def load_w(dram, name):
            t = wpool.tile([P, EC, E], bf16, name=name, tag=name)
            nc.sync.dma_start(out=t, in_=dram[:, :].rearrange("(c p) f -> p c f", p=P))
            return t

        def load_xb_chunk(b, ec, first=False):
            t = xpool.tile([P, S], bf16, name=f"xb{ec}", tag=f"xb{ec}")
            nc.sync.dma_start(
                out=t, in_=xT_d[b * E + ec * P : b * E + (ec + 1) * P, :]
            )
            return t

        wq_ec = []
        xb = []
        for ec in range(EC):
            t = wpool.tile([P, E], bf16, name=f"wq{ec}", tag=f"wq{ec}")
            nc.sync.dma_start(out=t, in_=wq_d[ec * P : (ec + 1) * P, :])
            wq_ec.append(t)
            xb.append(load_xb_chunk(0, ec, first=(ec == 0)))
        wk_ec = []
        for ec in range(EC):
            t = wpool.tile([P, E], bf16, name=f"wk{ec}", tag=f"wk{ec}")
            nc.sync.dma_start(out=t, in_=wk_d[ec * P : (ec + 1) * P, :])
            wk_ec.append(t)
        g_sb = consts.tile([P, 1], f32, tag="g")
        nc.sync.dma_start(out=g_sb, in_=gam_d[0:1, 0:1].to_broadcast([P, 1]))
        neg_g = consts.tile([P, 1], f32, tag="neg_g")
        nc.vector.tensor_scalar_mul(neg_g, g_sb, -1.0)
        two_g = consts.tile([P, 1], f32, tag="two_g")
        nc.vector.tensor_scalar_mul(two_g, g_sb, 2.0)
        wv = load_w(wv_d, "wv_sb")
        woT = load_w(woT_d, "woT_sb")

        for b in range(B):
            if b > 0:
                xb = [load_xb_chunk(b, ec) for ec in range(EC)]

            QT = qkpool.tile([P, FC, S], bf16, tag="QT")
            KT = qkpool.tile([P, FC, S], bf16, tag="KT")
            QTsq = sqpool.tile([P, FC, S], bf16, tag="QTsq")
            KTsq = sqpool.tile([P, FC, S], bf16, tag="KTsq")
            V = vpool.tile([P, TC, E], bf16, tag="V")
            A = apool.tile([P, TC, S], bf16, tag="A")
            OT = opool.tile([P, FC, S], bf16, tag="OT")
            c_sb = smalls.tile([P, TC], f32, tag="c")
            negk2 = smalls.tile([P, TC], f32, tag="negk2")

            # --- Q^T and K^T projections (+ squared copies for q2/k2) ---
            for wsel, OUT, OUTSQ in (("q", QT, QTsq), ("k", KT, KTsq)):
                for fc in range(FC):
                    pss = [psum.tile([P, NF], f32, tag="ps", name="ps") for _ in range(SH)]
                    for ec in range(EC):
                        w_ec = wq_ec if wsel == "q" else wk_ec
                        lhsT = w_ec[ec][:, fc * P : (fc + 1) * P]
                        for sh in range(SH):
                            nc.tensor.matmul(
                                pss[sh],
                                lhsT,
                                xb[ec][:, sh * NF : (sh + 1) * NF],
                                start=(ec == 0),
                                stop=(ec == EC - 1),
                            )
                    for sh in range(SH):
                        sl = slice(sh * NF, (sh + 1) * NF)
                        nc.vector.tensor_copy(out=OUT[:, fc, sl], in_=pss[sh])
                        nc.scalar.activation(out=OUTSQ[:, fc, sl], in_=pss[sh], func=Square)

            # --- V projection, natural [t, f] layout ---
            for t in range(TC):
                ps = psum.tile([P, NF], f32, tag="ps")
                for ec in range(EC):
                    nc.tensor.matmul(
                        ps,
                        xb[ec][:, t * P : (t + 1) * P],
                        wv[:, ec, :],
                        start=(ec == 0),
                        stop=(ec == EC - 1),
                    )
                nc.vector.tensor_copy(out=V[:, t, :], in_=ps)

            # --- q2/k2 fc pre-sum on (idle) GpSimd, in place into chunk 0 ---
            for SQ in (QTsq, KTsq):
                nc.gpsimd.tensor_add(SQ[:, 0, :], SQ[:, 0, :], SQ[:, 2, :])
                nc.gpsimd.tensor_add(SQ[:, 1, :], SQ[:, 1, :], SQ[:, 3, :])
                nc.gpsimd.tensor_add(SQ[:, 0, :], SQ[:, 0, :], SQ[:, 1, :])

            # --- scores S^T[t,s] = K Q^T, then A = exp(2g*S^T - g*k2[t]) ---
            # The q2/k2 cross-partition ones-matmuls are emitted after the
            # first score chunk so PE isn't stalled on the GpSimd pre-sums.
            for t in range(TC):
                pss = [psum.tile([P, NF], f32, tag="ps", name="ps") for _ in range(SH)]
                for fc in range(FC):
                    lhsT = KT[:, fc, t * P : (t + 1) * P]
                    for sh in range(SH):
                        nc.tensor.matmul(
                            pss[sh],
                            lhsT,
                            QT[:, fc, sh * NF : (sh + 1) * NF],
                            start=(fc == 0),
                            stop=(fc == FC - 1),
                        )
                psk = psum.tile([P, 1], f32, tag="ps", name="ps")
                nc.tensor.matmul(
                    psk, KTsq[:, 0, t * P : (t + 1) * P], ones, start=True, stop=True
                )
                nc.vector.tensor_scalar_mul(negk2[:, t : t + 1], psk, neg_g)
                psq = psum.tile([P, 1], f32, tag="ps", name="ps")
                nc.tensor.matmul(
                    psq, QTsq[:, 0, t * P : (t + 1) * P], ones, start=True, stop=True
                )
                nc.scalar.activation(
                    out=c_sb[:, t : t + 1], in_=psq, func=Exp, scale=neg_g
                )
                for sh in range(SH):
                    nc.scalar.activation(
                        out=A[:, t, sh * NF : (sh + 1) * NF],
                        in_=pss[sh],
                        func=Exp,
                        scale=two_g,
                        bias=negk2[:, t : t + 1],
                    )

            # --- OT[f,s] = sum_t V[t,f] A[t,s], then per s-half
            #     y[s,d] = (OT^T @ WoT) * exp(-g*q2[s]) ---
            # s-half outer so each half's Y matmuls start right after that
            # half's OT evictions instead of after the whole AV phase.
            for sh in range(SH):
                for fc in range(FC):
                    ps = psum.tile([P, NF], f32, tag="ps", name="ps")
                    for t in range(TC):
                        nc.tensor.matmul(
                            ps,
                            V[:, t, fc * P : (fc + 1) * P],
                            A[:, t, sh * NF : (sh + 1) * NF],
                            start=(t == 0),
                            stop=(t == TC - 1),
                        )
                    nc.vector.tensor_copy(
                        out=OT[:, fc, sh * NF : (sh + 1) * NF], in_=ps
                    )
                for j in range(sh * TC // SH, (sh + 1) * TC // SH):
                    ps = psum.tile([P, NF], f32, tag="ps", name="ps")
                    for fc in range(FC):
                        nc.tensor.matmul(
                            ps,
                            OT[:, fc, j * P : (j + 1) * P],
                            woT[:, fc, :],
                            start=(fc == 0),
                            stop=(fc == FC - 1),
                        )
                    ysb = ypool.tile([P, NF], f32, tag="ysb")
                    nc.scalar.activation(
                        out=ysb, in_=ps, func=Copy, scale=c_sb[:, j : j + 1]
                    )
                    nc.sync.dma_start(
                        out=y_d[b * S + j * P : b * S + (j + 1) * P, :], in_=ysb
                    )

    nc.compile()
    return nc


def _get_nc():
    if "nc" not in _CACHE:
        _CACHE["nc"] = _build_nc()
    return _CACHE["nc"]


def _make_in_maps(inputs):
    bf16 = ml_dtypes.bfloat16
    x = np.asarray(inputs["x"], np.float32)
    W_q = np.asarray(inputs["W_q"], np.float32)
    W_k = np.asarray(inputs["W_k"], np.float32)
    W_v = np.asarray(inputs["W_v"], np.float32)
    W_o = np.asarray(inputs["W_o"], np.float32)
    gamma = np.asarray(inputs["gamma"], np.float32).reshape(H)

    xT = np.ascontiguousarray(x.transpose(0, 2, 1)).reshape(B * E, S).astype(bf16)
    in_maps = []
    for h in range(H):
        in_maps.append(
            {
                "xT": xT,
                "wq": np.ascontiguousarray(W_q[h]).astype(bf16),
                "wk": np.ascontiguousarray(W_k[h]).astype(bf16),
                "wv": np.ascontiguousarray(W_v[h]).astype(bf16),
                "woT": np.ascontiguousarray(W_o[:, h * E : (h + 1) * E].T).astype(bf16),
                "gam": np.full((1, 1), gamma[h], np.float32),
            }
        )
    return in_maps


def run(inputs, trace=False, **kwargs):
    """Run on all 8 cores; returns (output, BassKernelResults)."""
    from concourse import bass_utils

    nc = _get_nc()
    res = bass_utils.run_bass_kernel_spmd(
        nc, _make_in_maps(inputs), core_ids=list(range(H)), trace=trace, **kwargs
    )
    out = np.zeros((B * S, E), np.float32)
    for r in res.results:
        out += r["y"]
    return out.reshape(B, S, E), res


def kernel(**inputs) -> np.ndarray:
    out, _ = run(inputs, trace=False)
    return out


# revision 23
# speedup vs baseline: 1.0093x; 1.0093x over previous
"""RBF-kernel attention (sparse_attention nn_Attention_31155692765862) on 8 trn2 cores.

Head-parallel sharding: core h computes head h for all batches:
    Q = x @ W_q[h], K = x @ W_k[h], V = x @ W_v[h]
    attn = exp(-gamma_h * max(q2 + k2 - 2 Q K^T, 0))
    y_h  = (attn @ V) @ W_o[:, h*E:(h+1)*E].T          (partial over heads)
Host sums the 8 partial y_h.

Device math (per batch b), all matmuls bf16 inputs with fp32 PSUM accum:
    QT[f,s] = sum_e Wq[e,f] xT[e,s]          (lhsT=Wq chunk, rhs=xT chunk)
    KT[f,s] likewise; V[t,f] = sum_e xT[e,t] Wv[e,f]
    q2[s] = sum_f QT[f,s]^2  via Square-eviction + ones-matmul (output [128,1])
    ST[t,s] = sum_f KT[f,t] QT[f,s]
    A[t,s] = exp(2g*ST - g*k2[t])            (ACT exp, per-partition bias)
    OT[f,s] = sum_t V[t,f] A[t,s]
    y[s,d] = (sum_f OT[f,s] WoT[f,d]) * exp(-g*q2[s])  (scale on eviction)
The relu clamp max(d2,0) is dropped: d2 < 0 only from rounding (|d2| ~ 1e-3
relative), so exp(-g*d2) differs from exp(0) by < 1e-4 relative.
"""

import numpy as np
import ml_dtypes
from contextlib import ExitStack

B, S, E, H, P = 4, 1024, 512, 8, 128
EC = E // P  # contraction chunks of 128 over the embedding dim
FC = E // P  # feature chunks of 128
TC = S // P  # row chunks of 128 over the sequence dim
NF = 512     # matmul moving free dim / PSUM bank width (fp32)
SH = S // NF  # s halves per row

_CACHE = {}


def _build_nc():
    import concourse.tile as tile
    from concourse import bacc, mybir

    bf16 = mybir.dt.bfloat16
    f32 = mybir.dt.float32
    Square = mybir.ActivationFunctionType.Square
    Exp = mybir.ActivationFunctionType.Exp
    Copy = mybir.ActivationFunctionType.Copy

    nc = bacc.Bacc("TRN2", target_bir_lowering=False, debug=False)

    xT_d = nc.dram_tensor("xT", [B * E, S], bf16, kind="ExternalInput")
    wq_d = nc.dram_tensor("wq", [E, E], bf16, kind="ExternalInput")
    wk_d = nc.dram_tensor("wk", [E, E], bf16, kind="ExternalInput")
    wv_d = nc.dram_tensor("wv", [E, E], bf16, kind="ExternalInput")
    woT_d = nc.dram_tensor("woT", [E, E], bf16, kind="ExternalInput")
    gam_d = nc.dram_tensor("gam", [1, 1], f32, kind="ExternalInput")
    y_d = nc.dram_tensor("y", [B * S, E], f32, kind="ExternalOutput")

    with tile.TileContext(nc) as tc, ExitStack() as ctx:
        consts = ctx.enter_context(tc.tile_pool(name="consts", bufs=1))
        wpool = ctx.enter_context(tc.tile_pool(name="wpool", bufs=1))
        xpool = ctx.enter_context(tc.tile_pool(name="xpool", bufs=2))
        qkpool = ctx.enter_context(tc.tile_pool(name="qkpool", bufs=2))
        sqpool = ctx.enter_context(tc.tile_pool(name="sqpool", bufs=2))
        vpool = ctx.enter_context(tc.tile_pool(name="vpool", bufs=2))
        apool = ctx.enter_context(tc.tile_pool(name="apool", bufs=2))
        opool = ctx.enter_context(tc.tile_pool(name="opool", bufs=2))
        ypool = ctx.enter_context(tc.tile_pool(name="ypool", bufs=4))
        smalls = ctx.enter_context(tc.tile_pool(name="smalls", bufs=2))
        psum = ctx.enter_context(tc.tile_pool(name="psum", bufs=8, space="PSUM"))

        # PE warmup: dummy matmuls on a zeroed tile bridge the initial DMA
        # wait and release the HAM clock throttle before real matmuls start.
        # Memsets go on GpSimd so nothing upstream delays them.
        warm_src = consts.tile([P, NF], bf16, tag="warm_src")
        nc.gpsimd.memset(warm_src, 0.0)
        ones = consts.tile([P, 1], bf16, tag="ones")
        nc.gpsimd.memset(ones, 1.0)
        warm_ps = psum.tile([P, NF], f32, tag="ps", name="warm_ps")
        for _ in range(8):
            nc.tensor.matmul(warm_ps, warm_src[:, :P], warm_src, start=True, stop=True)

        # Weights: [p, feat] per 128-row chunk of the embedding dim. wq is
        # split per chunk and interleaved with batch-0 x chunks so the first
        # projection's accumulation chain starts as soon as chunk 0 lands.
        def load_w(dram, name):
            t = wpool.tile([P, EC, E], bf16, name=name, tag=name)
            nc.sync.dma_start(out=t, in_=dram[:, :].rearrange("(c p) f -> p c f", p=P))
            return t

        def load_xb_chunk(b, ec, first=False):
            t = xpool.tile([P, S], bf16, name=f"xb{ec}", tag=f"xb{ec}")
            nc.sync.dma_start(
                out=t, in_=xT_d[b * E + ec * P : b * E + (ec + 1) * P, :]
            )
            return t

        wq_ec = []
        xb = []
        for ec in range(EC):
            t = wpool.tile([P, E], bf16, name=f"wq{ec}", tag=f"wq{ec}")
            nc.sync.dma_start(out=t, in_=wq_d[ec * P : (ec + 1) * P, :])
            wq_ec.append(t)
            xb.append(load_xb_chunk(0, ec, first=(ec == 0)))
        wk_ec = []
        for ec in range(EC):
            t = wpool.tile([P, E], bf16, name=f"wk{ec}", tag=f"wk{ec}")
            nc.sync.dma_start(out=t, in_=wk_d[ec * P : (ec + 1) * P, :])
            wk_ec.append(t)
        g_sb = consts.tile([P, 1], f32, tag="g")
        nc.sync.dma_start(out=g_sb, in_=gam_d[0:1, 0:1].to_broadcast([P, 1]))
        neg_g = consts.tile([P, 1], f32, tag="neg_g")
        nc.vector.tensor_scalar_mul(neg_g, g_sb, -1.0)
        two_g = consts.tile([P, 1], f32, tag="two_g")
        nc.vector.tensor_scalar_mul(two_g, g_sb, 2.0)
        wv = load_w(wv_d, "wv_sb")
        woT = load_w(woT_d, "woT_sb")

        for b in range(B):
            if b > 0:
                xb = [load_xb_chunk(b, ec) for ec in range(EC)]

            QT = qkpool.tile([P, FC, S], bf16, tag="QT")
            KT = qkpool.tile([P, FC, S], bf16, tag="KT")
            QTsq = sqpool.tile([P, FC, S], bf16, tag="QTsq")
            KTsq = sqpool.tile([P, FC, S], bf16, tag="KTsq")
            V = vpool.tile([P, TC, E], bf16, tag="V")
            A = apool.tile([P, TC, S], bf16, tag="A")
            OT = opool.tile([P, FC, S], bf16, tag="OT")
            c_sb = smalls.tile([P, TC], f32, tag="c")
            negk2 = smalls.tile([P, TC], f32, tag="negk2")

            # --- Q^T and K^T projections (+ squared copies for q2/k2) ---
            for wsel, OUT, OUTSQ in (("q", QT, QTsq), ("k", KT, KTsq)):
                for fc in range(FC):
                    pss = [psum.tile([P, NF], f32, tag="ps", name="ps") for _ in range(SH)]
                    for ec in range(EC):
                        w_ec = wq_ec if wsel == "q" else wk_ec
                        lhsT = w_ec[ec][:, fc * P : (fc + 1) * P]
                        for sh in range(SH):
                            nc.tensor.matmul(
                                pss[sh],
                                lhsT,
                                xb[ec][:, sh * NF : (sh + 1) * NF],
                                start=(ec == 0),
                                stop=(ec == EC - 1),
                            )
                    for sh in range(SH):
                        sl = slice(sh * NF, (sh + 1) * NF)
                        nc.vector.tensor_copy(out=OUT[:, fc, sl], in_=pss[sh])
                        nc.scalar.activation(out=OUTSQ[:, fc, sl], in_=pss[sh], func=Square)

            # --- V projection, natural [t, f] layout ---
            for t in range(TC):
                ps = psum.tile([P, NF], f32, tag="ps")
                for ec in range(EC):
                    nc.tensor.matmul(
                        ps,
                        xb[ec][:, t * P : (t + 1) * P],
                        wv[:, ec, :],
                        start=(ec == 0),
                        stop=(ec == EC - 1),
                    )
                nc.vector.tensor_copy(out=V[:, t, :], in_=ps)

            # --- q2/k2 fc pre-sum on (idle) GpSimd, in place into chunk 0 ---
            for SQ in (QTsq, KTsq):
                nc.gpsimd.tensor_add(SQ[:, 0, :], SQ[:, 0, :], SQ[:, 2, :])
                nc.gpsimd.tensor_add(SQ[:, 1, :], SQ[:, 1, :], SQ[:, 3, :])
                nc.gpsimd.tensor_add(SQ[:, 0, :], SQ[:, 0, :], SQ[:, 1, :])

            # --- scores S^T[t,s] = K Q^T, then A = exp(2g*S^T - g*k2[t]) ---
            # The q2/k2 cross-partition ones-matmuls are emitted after the
            # first score chunk so PE isn't stalled on the GpSimd pre-sums.
            for t in range(TC):
                pss = [psum.tile([P, NF], f32, tag="ps", name="ps") for _ in range(SH)]
                for fc in range(FC):
                    lhsT = KT[:, fc, t * P : (t + 1) * P]
                    for sh in range(SH):
                        nc.tensor.matmul(
                            pss[sh],
                            lhsT,
                            QT[:, fc, sh * NF : (sh + 1) * NF],
                            start=(fc == 0),
                            stop=(fc == FC - 1),
                        )
                psk = psum.tile([P, 1], f32, tag="ps", name="ps")
                nc.tensor.matmul(
                    psk, KTsq[:, 0, t * P : (t + 1) * P], ones, start=True, stop=True
                )
                nc.vector.tensor_scalar_mul(negk2[:, t : t + 1], psk, neg_g)
                psq = psum.tile([P, 1], f32, tag="ps", name="ps")
                nc.tensor.matmul(
                    psq, QTsq[:, 0, t * P : (t + 1) * P], ones, start=True, stop=True
                )
                nc.scalar.activation(
                    out=c_sb[:, t : t + 1], in_=psq, func=Exp, scale=neg_g
                )
                for sh in range(SH):
                    nc.scalar.activation(
                        out=A[:, t, sh * NF : (sh + 1) * NF],
                        in_=pss[sh],
                        func=Exp,
                        scale=two_g,
                        bias=negk2[:, t : t + 1],
                    )

            # --- OT[f,s] = sum_t V[t,f] A[t,s], then per s-half
            #     y[s,d] = (OT^T @ WoT) * exp(-g*q2[s]) ---
            # s-half outer so each half's Y matmuls start right after that
            # half's OT evictions instead of after the whole AV phase.
            for sh in range(SH):
                for fc in range(FC):
                    ps = psum.tile([P, NF], f32, tag="ps", name="ps")
                    for t in range(TC):
                        nc.tensor.matmul(
                            ps,
                            V[:, t, fc * P : (fc + 1) * P],
                            A[:, t, sh * NF : (sh + 1) * NF],
                            start=(t == 0),
                            stop=(t == TC - 1),
                        )
                    nc.vector.tensor_copy(
                        out=OT[:, fc, sh * NF : (sh + 1) * NF], in_=ps
                    )
                for j in range(sh * TC // SH, (sh + 1) * TC // SH):
                    ps = psum.tile([P, NF], f32, tag="ps", name="ps")
                    for fc in range(FC):
                        nc.tensor.matmul(
                            ps,
                            OT[:, fc, j * P : (j + 1) * P],
                            woT[:, fc, :],
                            start=(fc == 0),
                            stop=(fc == FC - 1),
                        )
                    ysb = ypool.tile([P, NF], f32, tag="ysb")
                    nc.scalar.activation(
                        out=ysb, in_=ps, func=Copy, scale=c_sb[:, j : j + 1]
                    )
                    nc.sync.dma_start(
                        out=y_d[b * S + j * P : b * S + (j + 1) * P, :], in_=ysb
                    )

    nc.compile()
    return nc


def _get_nc():
    if "nc" not in _CACHE:
        _CACHE["nc"] = _build_nc()
    return _CACHE["nc"]


def _make_in_maps(inputs):
    bf16 = ml_dtypes.bfloat16
    x = np.asarray(inputs["x"], np.float32)
    W_q = np.asarray(inputs["W_q"], np.float32)
    W_k = np.asarray(inputs["W_k"], np.float32)
    W_v = np.asarray(inputs["W_v"], np.float32)
    W_o = np.asarray(inputs["W_o"], np.float32)
    gamma = np.asarray(inputs["gamma"], np.float32).reshape(H)

    xT = np.ascontiguousarray(x.transpose(0, 2, 1)).reshape(B * E, S).astype(bf16)
    in_maps = []
    for h in range(H):
        in_maps.append(
            {
                "xT": xT,
                "wq": np.ascontiguousarray(W_q[h]).astype(bf16),
                "wk": np.ascontiguousarray(W_k[h]).astype(bf16),
                "wv": np.ascontiguousarray(W_v[h]).astype(bf16),
                "woT": np.ascontiguousarray(W_o[:, h * E : (h + 1) * E].T).astype(bf16),
                "gam": np.full((1, 1), gamma[h], np.float32),
            }
        )
    return in_maps


def run(inputs, trace=False, **kwargs):
    """Run on all 8 cores; returns (output, BassKernelResults)."""
    from concourse import bass_utils

    nc = _get_nc()
    res = bass_utils.run_bass_kernel_spmd(
        nc, _make_in_maps(inputs), core_ids=list(range(H)), trace=trace, **kwargs
    )
    out = np.zeros((B * S, E), np.float32)
    for r in res.results:
        out += r["y"]
    return out.reshape(B, S, E), res


def kernel(**inputs) -> np.ndarray:
    out, _ = run(inputs, trace=False)
    return out
